# revision 45
# baseline (speedup 1.0000x reference)
"""Trainium2 Bass kernel for nn_MultiHeadAttentionBlock_49967649521921.

Reference computation (per batch b, x viewed as [C=512, N=1024]):
    q = Wq @ x ; k = Wk @ x ; v = Wv @ x          (1x1 convs, biases are zeros)
    per head h (8 heads, hd=64):
      scores[d,e] = sum_n q_h[d,n] k_h[e,n] / 8
      attn = softmax(scores, axis=e)
      out_h[d,n]  = sum_e attn[d,e] v_h[e,n]
    y[c',s'] = out[h, d, n] with c' = h*64 + n//16, s' = (n%16)*64 + d
    final = Wo @ y    -> reshape [512, 32, 32]

Sharding: data-parallel over batch. 16 batches / 8 cores = 2 per core.
No collectives; host scatters inputs and gathers outputs.

Design (~89.7us vs the f32r baseline's ~153us):
  * everything bf16 on the matmul path (measured end-to-end max-rel-err
    5.7e-3 vs the 2e-2 gate): single-pass attention matmuls instead of
    fp32 LOW/HIGH double-pass, FWL weight loads, half the DMA bytes.
  * host permutes x's spatial columns n = a*16+r -> position r*64+a
    (projections and the q.k^T contraction are order-invariant), which
    makes attn@v's stationary v slices contiguous 64-column blocks AND
    lets the attn@v output land DIRECTLY in the transpose(2,3).reshape
    ("y") layout: y[h*64+a, r*64+d] = sum_e v_h[e, n=a*16+r] attn_h[d,e]
    -- one [64a x 64d] matmul per (head, r), no data scramble at all.
  * attn@v head pairs run in diagonal PE quadrants (tile_position (0,0)
    and (64,64)) so both streams execute concurrently.
  * scores per head-pair as 128-wide diagonal blocks; softmax over
    partitions via ones-matmul column sums + reciprocal + outer-product.
  * warmup matmuls at t0 keep the PE HAM clock-gate warm through the
    initial input-DMA window; the low-occupancy attn@v phases are
    interleaved with 512-free projection work so the clock-gate never
    re-throttles mid-kernel.
  * x fed in two host-side layouts (n-half-major for q/k, cc-major for
    v) so every input DMA moves 2-4KB contiguous rows; weights split
    across both HWDGE queues; output drained per half, the last oc in
    quarter-chunks on both copy engines and both queues.

Device-side layouts (per core):
  xa    2x [128, 2048] bf16  [p, (q2 cc n')] for q/k projections
  x_sb  [128, 4, 1024] bf16  channel-chunk-major view of x_b [C, N]
  qt/kt [128, 8, 512]  bf16  q^T/k^T (spatial on partitions)
  v_sb  [128, 4, 1024] bf16  [O, (r a)]
  y_sb  [128, 4, 1024] bf16  written directly by attn@v matmul drains
"""

import os
import sys

import numpy as np

for _p in ("/opt/trn_rl_repo",):
    if _p not in sys.path and os.path.isdir(_p):
        sys.path.insert(0, _p)

from contextlib import ExitStack

import ml_dtypes

import concourse.bass as bass
import concourse.tile as tile
from concourse import bacc
from concourse import mybir
from concourse.bass_utils import run_bass_kernel_spmd

F32 = mybir.dt.float32
BF16 = mybir.dt.bfloat16
AF = mybir.ActivationFunctionType

N_CORES = 8
B_PER_CORE = 2
C = 512
N = 1024
NH = 8
HD = 64

N_WARMUP = 13  # free-512 matmuls to keep the PE clock-gate warm at start


def _split_excess_dma_waits(nc):
    """walrus' static-DMA (PSEUDO_DMA_DIRECT2D) encoding accepts a single
    sync-wait; Bacc's generate_event_semaphores only splits waits on compute
    instructions. Move excess DMA waits onto preceding EventSemaphore
    carriers (2 waits each) on the same engine queue."""
    for f in nc.m.functions:
        for blk in f.blocks:
            changed = False
            new_insts = []
            for inst in blk.instructions:
                si = inst.sync_info
                waits = list(si.on_wait) if si is not None and si.on_wait else []
                if inst.opcode == "DMACopy" and len(waits) > 1:
                    keep, excess = waits[:1], waits[1:]
                    k = 0
                    while excess:
                        chunk, excess = excess[:2], excess[2:]
                        ev = mybir.InstEventSemaphore(
                            name=f"{inst.name}-evw{k}",
                            opcode="EventSemaphore",
                            engine=inst.engine,
                            sync_info=mybir.SyncInfo(on_wait=chunk, on_update=[]),
                        )
                        new_insts.append(ev)
                        k += 1
                    inst.sync_info = mybir.SyncInfo(
                        on_wait=keep, on_update=list(si.on_update or [])
                    )
                    changed = True
                new_insts.append(inst)
            if changed:
                blk.instructions = new_insts


def build_program():
    nc = bacc.Bacc("TRN2", target_bir_lowering=False, debug=False)

    # two host-side layouts of the same x: xa is n-half-major with each
    # partition's data contiguous (4KB DMA descriptors) for the q/k
    # projections; x (cc-major) feeds the v projection's 512-wide rhs.
    xa_d = nc.dram_tensor("xa", [B_PER_CORE, 2, 128, 2048], BF16, kind="ExternalInput").ap()
    x_d = nc.dram_tensor("x", [B_PER_CORE, C, N], BF16, kind="ExternalInput").ap()
    wq_d = nc.dram_tensor("wqt", [C, C], BF16, kind="ExternalInput").ap()
    wk_d = nc.dram_tensor("wkt", [C, C], BF16, kind="ExternalInput").ap()
    wv_d = nc.dram_tensor("wvt", [C, C], BF16, kind="ExternalInput").ap()
    wo_d = nc.dram_tensor("wot", [C, C], BF16, kind="ExternalInput").ap()
    out_d = nc.dram_tensor("out", [B_PER_CORE, C, N], F32, kind="ExternalOutput").ap()

    with tile.TileContext(nc) as tc, ExitStack() as ctx, nc.allow_low_precision(
        reason="bf16 kernel; end-to-end max-rel-err 5.5e-3 vs 2e-2 budget"
    ):
        wp = ctx.enter_context(tc.tile_pool(name="w", bufs=1))
        xp = ctx.enter_context(tc.tile_pool(name="x", bufs=2))
        qkp = ctx.enter_context(tc.tile_pool(name="qk", bufs=1))
        vp = ctx.enter_context(tc.tile_pool(name="v", bufs=2))
        yp = ctx.enter_context(tc.tile_pool(name="y", bufs=2))
        smp = ctx.enter_context(tc.tile_pool(name="sm", bufs=2))
        ogp = ctx.enter_context(tc.tile_pool(name="og", bufs=2))
        cst = ctx.enter_context(tc.tile_pool(name="cst", bufs=1))

        ps_big = ctx.enter_context(tc.tile_pool(name="psb", bufs=5, space="PSUM"))
        ps_sc = ctx.enter_context(tc.tile_pool(name="pss", bufs=2, space="PSUM"))
        ps_sm = ctx.enter_context(tc.tile_pool(name="psm", bufs=1, space="PSUM"))

        # warmup operand (zeros) -- issued before any data-dependent matmul
        wm = cst.tile([128, 512], BF16)
        nc.vector.memset(wm[:, :], 0.0)

        # constants
        ones_lo = cst.tile([128, 1], BF16)
        nc.vector.memset(ones_lo[:, :], 0.0)
        nc.vector.memset(ones_lo[0:64, :], 1.0)
        ones_hi = cst.tile([128, 1], BF16)
        nc.vector.memset(ones_hi[:, :], 0.0)
        nc.vector.memset(ones_hi[64:128, :], 1.0)
        ones_row = cst.tile([1, 128], BF16)
        nc.vector.memset(ones_row[:, :], 1.0)

        # PE warmup: keep the array busy during the initial input DMA so
        # the HAM clock-gate is at 8/8 when real matmuls arrive.
        for i in range(N_WARMUP):
            pw = ps_big.tile([128, 512], F32, tag="big", name=f"warm{i}")
            nc.tensor.matmul(pw[:, :], wm[:, 0:128], wm[:, :],
                             start=True, stop=True)

        w_sb = {}

        def _load_w(name, d, eng1, eng2):
            t = wp.tile([128, 4, C], BF16, tag=name, name=f"w_{name}")
            dr = d.rearrange("(cc p) o -> p cc o", p=128)
            eng1.dma_start(t[:, 0:2, :], dr[:, 0:2, :])
            eng2.dma_start(t[:, 2:4, :], dr[:, 2:4, :])
            w_sb[name] = t

        st = [{} for _ in range(B_PER_CORE)]

        def s_load_xa(b, split):
            xa = []
            for h in range(2):
                t = xp.tile([128, 2048], BF16, tag=f"xa{h}", name=f"xa{b}_{h}")
                if split:
                    # both queues per half: the first n-quarter (and with it
                    # the first q projection chunk) is ready ~1us after wq
                    nc.sync.dma_start(t[:, 0:1024], xa_d[b, h, :, 0:1024])
                    nc.scalar.dma_start(t[:, 1024:2048], xa_d[b, h, :, 1024:2048])
                else:
                    eng = nc.sync if h == 0 else nc.scalar
                    eng.dma_start(t[:, :], xa_d[b, h])
                xa.append(t)
            st[b]["xa"] = xa

        def s_load_xb(b):
            x_sb = xp.tile([128, 4, N], BF16, tag="xsb", name=f"x_sb{b}")
            xr = x_d[b].rearrange("(cc p) n -> p cc n", p=128)
            nc.sync.dma_start(x_sb[:, 0:2, :], xr[:, 0:2, :])
            nc.scalar.dma_start(x_sb[:, 2:4, :], xr[:, 2:4, :])
            st[b]["x"] = x_sb

        def s_proj_qk_alloc(b):
            qt_sb = qkp.tile([128, 8, 512], BF16, tag="qt", name=f"qt{b}")
            kt_sb = qkp.tile([128, 8, 512], BF16, tag="kt", name=f"kt{b}")
            st[b]["qt"], st[b]["kt"] = qt_sb, kt_sb

        def s_proj_qk_chunk(b, ncn, which):
            # q and k matmuls share the stationary x chunk: issuing them
            # back-to-back per cc lets the weight load be reused/overlapped.
            xa = st[b]["xa"][ncn // 4]
            c0 = ((ncn % 4) // 2) * 1024 + (ncn % 2) * 128
            if which == "qk":
                s_proj_qk_chunk(b, ncn, "q")
                s_proj_qk_chunk(b, ncn, "k")
            elif which == "q":
                pq = ps_big.tile([128, 512], F32, tag="big", name=f"pq{b}_{ncn}")
                for cc in range(4):
                    nc.tensor.matmul(
                        pq[:, :], xa[:, c0 + cc * 256 : c0 + cc * 256 + 128],
                        w_sb["wq"][:, cc, :],
                        start=(cc == 0), stop=(cc == 3),
                    )
                nc.vector.tensor_copy(st[b]["qt"][:, ncn, :], pq[:, :])
            else:
                pk = ps_big.tile([128, 512], F32, tag="big", name=f"pk{b}_{ncn}")
                for cc in range(4):
                    nc.tensor.matmul(
                        pk[:, :], xa[:, c0 + cc * 256 : c0 + cc * 256 + 128],
                        w_sb["wk"][:, cc, :],
                        start=(cc == 0), stop=(cc == 3),
                    )
                nc.scalar.copy(st[b]["kt"][:, ncn, :], pk[:, :])

        def s_proj_qk(b):
            s_proj_qk_alloc(b)
            # q chunks 0-3 first: they only need wq + the first xa half; by
            # the time they finish streaming, wk has landed for the k chunks.
            for ncn in range(4):
                s_proj_qk_chunk(b, ncn, "q")
            for ncn in range(4):
                s_proj_qk_chunk(b, ncn, "k")
            for ncn in range(4, 8):
                s_proj_qk_chunk(b, ncn, "qk")

        def s_scores(b):
            qt_sb, kt_sb = st[b]["qt"], st[b]["kt"]
            et = smp.tile([128, 4, HD], BF16, tag="et", name=f"et{b}")
            for p in range(4):
                psl = slice(p * 128, (p + 1) * 128)
                ps_s = ps_sc.tile([128, 128], F32, tag="scores", name=f"ps_s{b}_{p}")
                for ncn in range(8):
                    nc.tensor.matmul(
                        ps_s[:, :],
                        kt_sb[:, ncn, psl],
                        qt_sb[:, ncn, psl],
                        start=(ncn == 0), stop=(ncn == 7),
                    )
                for hh in range(2):
                    s0 = hh * 64
                    nc.scalar.activation(
                        et[s0 : s0 + 64, p, :],
                        ps_s[s0 : s0 + 64, s0 : s0 + 64],
                        AF.Exp, scale=0.125,
                    )
            ps_r = ps_sm.tile([1, 512], F32, tag="small", name=f"ps_r{b}")
            for p in range(4):
                for hh in range(2):
                    h = 2 * p + hh
                    nc.tensor.matmul(
                        ps_r[0:1, h * 64 : (h + 1) * 64],
                        (ones_lo if hh == 0 else ones_hi)[:, 0:1],
                        et[:, p, :],
                        start=True, stop=True,
                    )
            recip = smp.tile([1, 512], BF16, tag="recip", name=f"recip{b}")
            nc.vector.reciprocal(recip[0:1, :], ps_r[0:1, :])
            st[b]["et"], st[b]["recip"] = et, recip

        def s_proj_v(b):
            x_sb = st[b]["x"]
            v_sb = vp.tile([128, 4, N], BF16, tag="vsb", name=f"v_sb{b}")
            for oc in range(4):
                for nh in range(2):
                    pv = ps_big.tile([128, 512], F32, tag="big", name=f"pv{b}_{oc}_{nh}")
                    for cc in range(4):
                        nc.tensor.matmul(
                            pv[:, :],
                            w_sb["wv"][:, cc, oc * 128 : (oc + 1) * 128],
                            x_sb[:, cc, nh * 512 : (nh + 1) * 512],
                            start=(cc == 0), stop=(cc == 3),
                        )
                    if nh == 0:
                        nc.vector.tensor_copy(v_sb[:, oc, 0:512], pv[:, :])
                    else:
                        nc.scalar.copy(v_sb[:, oc, 512:1024], pv[:, :])
            st[b]["v"] = v_sb

        def s_attn_norm(b):
            et, recip = st[b]["et"], st[b]["recip"]
            at = smp.tile([128, 4, HD], BF16, tag="at", name=f"at{b}")
            ps_rep = ps_big.tile([128, 512], F32, tag="big", name=f"ps_rep{b}")
            nc.tensor.matmul(
                ps_rep[:, :],
                ones_row[0:1, :],
                recip[0:1, :],
                start=True, stop=True,
            )
            for h in range(NH):
                s0 = (h % 2) * 64
                nc.vector.tensor_mul(
                    at[s0 : s0 + 64, h // 2, :],
                    et[s0 : s0 + 64, h // 2, :],
                    ps_rep[s0 : s0 + 64, h * 64 : (h + 1) * 64],
                )
            st[b]["at"] = at
            y_sb = yp.tile([128, 4, N], BF16, tag="ysb", name=f"y_sb{b}")
            st[b]["y"] = y_sb

        def s_outT_pair(b, hp):
            """attn@v for head pair (2*hp, 2*hp+1), computed directly in the
            scrambled y layout: y[h*64+a, r*64+d] = sum_e v_h[e, a*16+r] *
            attn_h[d, e]. For each r (= n%16), lhsT is the column-strided v
            slice [e, a] and rhs is attn^T [e, d]; the matmul output block
            [64(a), 64(d)] IS a y tile. Even head runs in PE quadrant (0,0),
            odd head in (64,64), so the pair executes concurrently."""
            at, v_sb = st[b]["at"], st[b]["v"]
            y_sb = st[b]["y"]
            po_y = [
                ps_big.tile([128, 512], F32, tag="big", name=f"po{b}_{hp}_{sh}")
                for sh in range(2)
            ]
            # x columns are host-permuted to (r a) order, so v_sb's 64-column
            # block r*64:(r+1)*64 is exactly v_h[e, a*16+r] for a=0..63 --
            # contiguous, hence FWL-eligible weight loads.
            for r in range(16):
                sh, rr = r // 8, r % 8
                for hh in range(2):
                    s0 = hh * 64
                    nc.tensor.matmul(
                        po_y[sh][s0 : s0 + 64, rr * 64 : (rr + 1) * 64],
                        v_sb[s0 : s0 + 64, hp, r * 64 : (r + 1) * 64],
                        at[s0 : s0 + 64, hp, :],
                        start=True, stop=True,
                        tile_position=(s0, s0),
                    )
            nc.vector.tensor_copy(y_sb[:, hp, 0:512], po_y[0][:, :])
            nc.scalar.copy(y_sb[:, hp, 512:1024], po_y[1][:, :])

        def s_final_oc(b, oc, fine=False):
            y_sb = st[b]["y"]
            og = ogp.tile([128, N], F32, tag="og", name=f"og{b}_{oc}")
            osl = slice(oc * 128, (oc + 1) * 128)
            # both n-halves share the stationary wo chunk per cp; the two
            # PSUM groups close together so the copies drain in parallel
            pf = [
                ps_big.tile([128, 512], F32, tag="big", name=f"pf{b}_{oc}_{sh}")
                for sh in range(2)
            ]
            for cp in range(4):
                wc = w_sb["wo"][:, cp, oc * 128 : (oc + 1) * 128]
                for sh in range(2):
                    nc.tensor.matmul(
                        pf[sh][:, :], wc,
                        y_sb[:, cp, sh * 512 : (sh + 1) * 512],
                        start=(cp == 0), stop=(cp == 3),
                    )
            for sh in range(2):
                if not fine:
                    sl = slice(sh * 512, (sh + 1) * 512)
                    if sh == 0:
                        nc.vector.tensor_copy(og[:, sl], pf[sh][:, :])
                        nc.sync.dma_start(out_d[b, osl, sl], og[:, sl])
                    else:
                        nc.scalar.copy(og[:, sl], pf[sh][:, :])
                        nc.scalar.dma_start(out_d[b, osl, sl], og[:, sl])
                else:
                    # drain the kernel's last output with both copy engines
                    # and both DMA queues working in parallel: vector+sync
                    # take sh0, scalar+scalar-queue take sh1 (one trigger
                    # each, issued immediately after its copy)
                    sl = slice(sh * 512, (sh + 1) * 512)
                    if sh == 0:
                        nc.vector.tensor_copy(og[:, sl], pf[sh][:, :])
                        nc.sync.dma_start(out_d[b, osl, sl], og[:, sl])
                    else:
                        nc.scalar.copy(og[:, sl], pf[sh][:, :])
                        nc.scalar.dma_start(out_d[b, osl, sl], og[:, sl])

        # input loads, ordered by first use; every item split across both
        # HWDGE queues so the first q matmul starts ~8us and the projection
        # stream never starves.
        _load_w("wq", wq_d, nc.sync, nc.scalar)
        s_load_xa(0, split=True)
        _load_w("wk", wk_d, nc.sync, nc.scalar)
        s_load_xb(0)
        _load_w("wv", wv_d, nc.sync, nc.scalar)
        s_load_xa(1, split=False)
        s_load_xb(1)
        _load_w("wo", wo_d, nc.sync, nc.scalar)

        # two-batch software pipeline. attn@v phases (64-free matmuls, low
        # PE-array occupancy) are interleaved with 512-free projection work
        # so the HAM clock-gate never sees an idle window.
        s_proj_qk(0)
        s_scores(0)
        s_proj_v(0)
        s_attn_norm(0)
        s_proj_qk_alloc(1)
        for hp in range(4):
            s_outT_pair(0, hp)
            s_proj_qk_chunk(1, 2 * hp, "qk")
            s_proj_qk_chunk(1, 2 * hp + 1, "qk")
        s_scores(1)
        s_proj_v(1)
        s_attn_norm(1)
        for hp in range(4):
            s_outT_pair(1, hp)
            s_final_oc(0, hp)
        for oc in range(4):
            s_final_oc(1, oc, fine=(oc == 3))

    nc.compile()
    _split_excess_dma_waits(nc)
    return nc


_PROGRAM = None


def _get_program():
    global _PROGRAM
    if _PROGRAM is None:
        _PROGRAM = build_program()
    return _PROGRAM


def make_in_maps(x, Wq, Wk, Wv, Wo):
    bf = ml_dtypes.bfloat16
    x, Wq, Wk, Wv, Wo = (np.asarray(a, dtype=np.float32) for a in (x, Wq, Wk, Wv, Wo))
    # Permute spatial columns n = a*16+r -> position r*64+a. The q/k/v
    # projections and the q.k^T contraction are invariant to spatial order;
    # the permutation makes v_sb r-major so each attn@v stationary operand
    # is a contiguous 64-column block.
    x = x.reshape(16, C, 64, 16).transpose(0, 1, 3, 2).reshape(16, C, N)
    x = x.astype(bf)
    # xa: n-half-major layout [b, half, p, (q2 cc n')] -- per-partition
    # contiguous 4KB rows for the q/k projection loads.
    xa = np.ascontiguousarray(
        x.reshape(16, 4, 128, 2, 2, 256).transpose(0, 3, 2, 4, 1, 5).reshape(16, 2, 128, 2048)
    )
    x = np.ascontiguousarray(x)
    wqt = np.ascontiguousarray(Wq.T.astype(bf))
    wkt = np.ascontiguousarray(Wk.T.astype(bf))
    wvt = np.ascontiguousarray(Wv.T.astype(bf))
    wot = np.ascontiguousarray(Wo.T.astype(bf))
    in_maps = []
    for c in range(N_CORES):
        bs = slice(c * B_PER_CORE, (c + 1) * B_PER_CORE)
        in_maps.append(
            {
                "x": np.ascontiguousarray(x[bs]),
                "xa": np.ascontiguousarray(xa[bs]),
                "wqt": wqt,
                "wkt": wkt,
                "wvt": wvt,
                "wot": wot,
            }
        )
    return in_maps


def kernel(x, Wq, bq, Wk, bk, Wv, bv, Wo, bo, _trace=False):
    # biases are zeros by construction in this problem (spec fill="zeros");
    # they are not applied on-device.
    nc = _get_program()
    in_maps = make_in_maps(x, Wq, Wk, Wv, Wo)
    res = run_bass_kernel_spmd(nc, in_maps, list(range(N_CORES)), trace=_trace)
    outs = [np.asarray(res.results[c]["out"]) for c in range(N_CORES)]
    full = np.concatenate(outs, axis=0).reshape(16, C, 32, 32)
    if _trace:
        return full, res
    return full


# revision 46
# speedup vs baseline: 1.0174x; 1.0174x over previous
"""Trainium2 Bass kernel for nn_MultiHeadAttentionBlock_49967649521921.

Reference computation (per batch b, x viewed as [C=512, N=1024]):
    q = Wq @ x ; k = Wk @ x ; v = Wv @ x          (1x1 convs, biases are zeros)
    per head h (8 heads, hd=64):
      scores[d,e] = sum_n q_h[d,n] k_h[e,n] / 8
      attn = softmax(scores, axis=e)
      out_h[d,n]  = sum_e attn[d,e] v_h[e,n]
    y[c',s'] = out[h, d, n] with c' = h*64 + n//16, s' = (n%16)*64 + d
    final = Wo @ y    -> reshape [512, 32, 32]

Sharding: data-parallel over batch. 16 batches / 8 cores = 2 per core.
No collectives; host scatters inputs and gathers outputs.

Design (~89.7us vs the f32r baseline's ~153us):
  * everything bf16 on the matmul path (measured end-to-end max-rel-err
    5.7e-3 vs the 2e-2 gate): single-pass attention matmuls instead of
    fp32 LOW/HIGH double-pass, FWL weight loads, half the DMA bytes.
  * host permutes x's spatial columns n = a*16+r -> position r*64+a
    (projections and the q.k^T contraction are order-invariant), which
    makes attn@v's stationary v slices contiguous 64-column blocks AND
    lets the attn@v output land DIRECTLY in the transpose(2,3).reshape
    ("y") layout: y[h*64+a, r*64+d] = sum_e v_h[e, n=a*16+r] attn_h[d,e]
    -- one [64a x 64d] matmul per (head, r), no data scramble at all.
  * attn@v head pairs run in diagonal PE quadrants (tile_position (0,0)
    and (64,64)) so both streams execute concurrently.
  * scores per head-pair as 128-wide diagonal blocks; softmax over
    partitions via ones-matmul column sums + reciprocal + outer-product.
  * warmup matmuls at t0 keep the PE HAM clock-gate warm through the
    initial input-DMA window; the low-occupancy attn@v phases are
    interleaved with 512-free projection work so the clock-gate never
    re-throttles mid-kernel.
  * x fed in two host-side layouts (n-half-major for q/k, cc-major for
    v) so every input DMA moves 2-4KB contiguous rows; weights split
    across both HWDGE queues; output drained per half, the last oc in
    quarter-chunks on both copy engines and both queues.

Device-side layouts (per core):
  xa    2x [128, 2048] bf16  [p, (q2 cc n')] for q/k projections
  x_sb  [128, 4, 1024] bf16  channel-chunk-major view of x_b [C, N]
  qt/kt [128, 8, 512]  bf16  q^T/k^T (spatial on partitions)
  v_sb  [128, 4, 1024] bf16  [O, (r a)]
  y_sb  [128, 4, 1024] bf16  written directly by attn@v matmul drains
"""

import os
import sys

import numpy as np

for _p in ("/opt/trn_rl_repo",):
    if _p not in sys.path and os.path.isdir(_p):
        sys.path.insert(0, _p)

from contextlib import ExitStack

import ml_dtypes

import concourse.bass as bass
import concourse.tile as tile
from concourse import bacc
from concourse import mybir
from concourse.bass_utils import run_bass_kernel_spmd

F32 = mybir.dt.float32
BF16 = mybir.dt.bfloat16
AF = mybir.ActivationFunctionType

N_CORES = 8
B_PER_CORE = 2
C = 512
N = 1024
NH = 8
HD = 64

N_WARMUP = 13  # free-512 matmuls to keep the PE clock-gate warm at start


def _split_excess_dma_waits(nc):
    """walrus' static-DMA (PSEUDO_DMA_DIRECT2D) encoding accepts a single
    sync-wait; Bacc's generate_event_semaphores only splits waits on compute
    instructions. Move excess DMA waits onto preceding EventSemaphore
    carriers (2 waits each) on the same engine queue."""
    for f in nc.m.functions:
        for blk in f.blocks:
            changed = False
            new_insts = []
            for inst in blk.instructions:
                si = inst.sync_info
                waits = list(si.on_wait) if si is not None and si.on_wait else []
                if inst.opcode == "DMACopy" and len(waits) > 1:
                    keep, excess = waits[:1], waits[1:]
                    k = 0
                    while excess:
                        chunk, excess = excess[:2], excess[2:]
                        ev = mybir.InstEventSemaphore(
                            name=f"{inst.name}-evw{k}",
                            opcode="EventSemaphore",
                            engine=inst.engine,
                            sync_info=mybir.SyncInfo(on_wait=chunk, on_update=[]),
                        )
                        new_insts.append(ev)
                        k += 1
                    inst.sync_info = mybir.SyncInfo(
                        on_wait=keep, on_update=list(si.on_update or [])
                    )
                    changed = True
                new_insts.append(inst)
            if changed:
                blk.instructions = new_insts


def build_program():
    nc = bacc.Bacc("TRN2", target_bir_lowering=False, debug=False)

    # two host-side layouts of the same x: xa is n-half-major with each
    # partition's data contiguous (4KB DMA descriptors) for the q/k
    # projections; x (cc-major) feeds the v projection's 512-wide rhs.
    xa_d = nc.dram_tensor("xa", [B_PER_CORE, 2, 128, 2048], BF16, kind="ExternalInput").ap()
    x_d = nc.dram_tensor("x", [B_PER_CORE, C, N], BF16, kind="ExternalInput").ap()
    wq_d = nc.dram_tensor("wqt", [C, C], BF16, kind="ExternalInput").ap()
    wk_d = nc.dram_tensor("wkt", [C, C], BF16, kind="ExternalInput").ap()
    wv_d = nc.dram_tensor("wvt", [C, C], BF16, kind="ExternalInput").ap()
    wo_d = nc.dram_tensor("wot", [C, C], BF16, kind="ExternalInput").ap()
    out_d = nc.dram_tensor("out", [B_PER_CORE, C, N], F32, kind="ExternalOutput").ap()

    with tile.TileContext(nc) as tc, ExitStack() as ctx, nc.allow_low_precision(
        reason="bf16 kernel; end-to-end max-rel-err 5.5e-3 vs 2e-2 budget"
    ):
        wp = ctx.enter_context(tc.tile_pool(name="w", bufs=1))
        xp = ctx.enter_context(tc.tile_pool(name="x", bufs=2))
        qkp = ctx.enter_context(tc.tile_pool(name="qk", bufs=1))
        vp = ctx.enter_context(tc.tile_pool(name="v", bufs=2))
        yp = ctx.enter_context(tc.tile_pool(name="y", bufs=2))
        smp = ctx.enter_context(tc.tile_pool(name="sm", bufs=2))
        # 4 bufs: with 2, the last og tile reuses a buffer whose output DMA
        # is still draining, stalling the final copies ~2.3us (measured)
        ogp = ctx.enter_context(tc.tile_pool(name="og", bufs=4))
        cst = ctx.enter_context(tc.tile_pool(name="cst", bufs=1))

        ps_big = ctx.enter_context(tc.tile_pool(name="psb", bufs=5, space="PSUM"))
        ps_sc = ctx.enter_context(tc.tile_pool(name="pss", bufs=2, space="PSUM"))
        ps_sm = ctx.enter_context(tc.tile_pool(name="psm", bufs=1, space="PSUM"))

        # warmup operand (zeros) -- issued before any data-dependent matmul
        wm = cst.tile([128, 512], BF16)
        nc.vector.memset(wm[:, :], 0.0)

        # constants
        ones_lo = cst.tile([128, 1], BF16)
        nc.vector.memset(ones_lo[:, :], 0.0)
        nc.vector.memset(ones_lo[0:64, :], 1.0)
        ones_hi = cst.tile([128, 1], BF16)
        nc.vector.memset(ones_hi[:, :], 0.0)
        nc.vector.memset(ones_hi[64:128, :], 1.0)
        ones_row = cst.tile([1, 128], BF16)
        nc.vector.memset(ones_row[:, :], 1.0)

        # PE warmup: keep the array busy during the initial input DMA so
        # the HAM clock-gate is at 8/8 when real matmuls arrive.
        for i in range(N_WARMUP):
            pw = ps_big.tile([128, 512], F32, tag="big", name=f"warm{i}")
            nc.tensor.matmul(pw[:, :], wm[:, 0:128], wm[:, :],
                             start=True, stop=True)

        w_sb = {}

        def _load_w(name, d, eng1, eng2):
            t = wp.tile([128, 4, C], BF16, tag=name, name=f"w_{name}")
            dr = d.rearrange("(cc p) o -> p cc o", p=128)
            eng1.dma_start(t[:, 0:2, :], dr[:, 0:2, :])
            eng2.dma_start(t[:, 2:4, :], dr[:, 2:4, :])
            w_sb[name] = t

        st = [{} for _ in range(B_PER_CORE)]

        def s_load_xa(b, split):
            xa = []
            for h in range(2):
                t = xp.tile([128, 2048], BF16, tag=f"xa{h}", name=f"xa{b}_{h}")
                if split:
                    # both queues per half: the first n-quarter (and with it
                    # the first q projection chunk) is ready ~1us after wq
                    nc.sync.dma_start(t[:, 0:1024], xa_d[b, h, :, 0:1024])
                    nc.scalar.dma_start(t[:, 1024:2048], xa_d[b, h, :, 1024:2048])
                else:
                    eng = nc.sync if h == 0 else nc.scalar
                    eng.dma_start(t[:, :], xa_d[b, h])
                xa.append(t)
            st[b]["xa"] = xa

        def s_load_xb(b):
            x_sb = xp.tile([128, 4, N], BF16, tag="xsb", name=f"x_sb{b}")
            xr = x_d[b].rearrange("(cc p) n -> p cc n", p=128)
            nc.sync.dma_start(x_sb[:, 0:2, :], xr[:, 0:2, :])
            nc.scalar.dma_start(x_sb[:, 2:4, :], xr[:, 2:4, :])
            st[b]["x"] = x_sb

        def s_proj_qk_alloc(b):
            qt_sb = qkp.tile([128, 8, 512], BF16, tag="qt", name=f"qt{b}")
            kt_sb = qkp.tile([128, 8, 512], BF16, tag="kt", name=f"kt{b}")
            st[b]["qt"], st[b]["kt"] = qt_sb, kt_sb

        def s_proj_qk_chunk(b, ncn, which):
            # q and k matmuls share the stationary x chunk: issuing them
            # back-to-back per cc lets the weight load be reused/overlapped.
            xa = st[b]["xa"][ncn // 4]
            c0 = ((ncn % 4) // 2) * 1024 + (ncn % 2) * 128
            if which == "qk":
                s_proj_qk_chunk(b, ncn, "q")
                s_proj_qk_chunk(b, ncn, "k")
            elif which == "q":
                pq = ps_big.tile([128, 512], F32, tag="big", name=f"pq{b}_{ncn}")
                for cc in range(4):
                    nc.tensor.matmul(
                        pq[:, :], xa[:, c0 + cc * 256 : c0 + cc * 256 + 128],
                        w_sb["wq"][:, cc, :],
                        start=(cc == 0), stop=(cc == 3),
                    )
                nc.vector.tensor_copy(st[b]["qt"][:, ncn, :], pq[:, :])
            else:
                pk = ps_big.tile([128, 512], F32, tag="big", name=f"pk{b}_{ncn}")
                for cc in range(4):
                    nc.tensor.matmul(
                        pk[:, :], xa[:, c0 + cc * 256 : c0 + cc * 256 + 128],
                        w_sb["wk"][:, cc, :],
                        start=(cc == 0), stop=(cc == 3),
                    )
                nc.scalar.copy(st[b]["kt"][:, ncn, :], pk[:, :])

        def s_proj_qk(b):
            s_proj_qk_alloc(b)
            # q chunks 0-3 first: they only need wq + the first xa half; by
            # the time they finish streaming, wk has landed for the k chunks.
            for ncn in range(4):
                s_proj_qk_chunk(b, ncn, "q")
            for ncn in range(4):
                s_proj_qk_chunk(b, ncn, "k")
            for ncn in range(4, 8):
                s_proj_qk_chunk(b, ncn, "qk")

        def s_scores(b):
            qt_sb, kt_sb = st[b]["qt"], st[b]["kt"]
            et = smp.tile([128, 4, HD], BF16, tag="et", name=f"et{b}")
            for p in range(4):
                psl = slice(p * 128, (p + 1) * 128)
                ps_s = ps_sc.tile([128, 128], F32, tag="scores", name=f"ps_s{b}_{p}")
                for ncn in range(8):
                    nc.tensor.matmul(
                        ps_s[:, :],
                        kt_sb[:, ncn, psl],
                        qt_sb[:, ncn, psl],
                        start=(ncn == 0), stop=(ncn == 7),
                    )
                for hh in range(2):
                    s0 = hh * 64
                    nc.scalar.activation(
                        et[s0 : s0 + 64, p, :],
                        ps_s[s0 : s0 + 64, s0 : s0 + 64],
                        AF.Exp, scale=0.125,
                    )
            ps_r = ps_sm.tile([1, 512], F32, tag="small", name=f"ps_r{b}")
            for p in range(4):
                for hh in range(2):
                    h = 2 * p + hh
                    nc.tensor.matmul(
                        ps_r[0:1, h * 64 : (h + 1) * 64],
                        (ones_lo if hh == 0 else ones_hi)[:, 0:1],
                        et[:, p, :],
                        start=True, stop=True,
                    )
            recip = smp.tile([1, 512], BF16, tag="recip", name=f"recip{b}")
            nc.vector.reciprocal(recip[0:1, :], ps_r[0:1, :])
            st[b]["et"], st[b]["recip"] = et, recip

        def s_proj_v(b):
            x_sb = st[b]["x"]
            v_sb = vp.tile([128, 4, N], BF16, tag="vsb", name=f"v_sb{b}")
            for oc in range(4):
                for nh in range(2):
                    pv = ps_big.tile([128, 512], F32, tag="big", name=f"pv{b}_{oc}_{nh}")
                    for cc in range(4):
                        nc.tensor.matmul(
                            pv[:, :],
                            w_sb["wv"][:, cc, oc * 128 : (oc + 1) * 128],
                            x_sb[:, cc, nh * 512 : (nh + 1) * 512],
                            start=(cc == 0), stop=(cc == 3),
                        )
                    if nh == 0:
                        nc.vector.tensor_copy(v_sb[:, oc, 0:512], pv[:, :])
                    else:
                        nc.scalar.copy(v_sb[:, oc, 512:1024], pv[:, :])
            st[b]["v"] = v_sb

        def s_attn_norm(b):
            et, recip = st[b]["et"], st[b]["recip"]
            at = smp.tile([128, 4, HD], BF16, tag="at", name=f"at{b}")
            ps_rep = ps_big.tile([128, 512], F32, tag="big", name=f"ps_rep{b}")
            nc.tensor.matmul(
                ps_rep[:, :],
                ones_row[0:1, :],
                recip[0:1, :],
                start=True, stop=True,
            )
            for h in range(NH):
                s0 = (h % 2) * 64
                nc.vector.tensor_mul(
                    at[s0 : s0 + 64, h // 2, :],
                    et[s0 : s0 + 64, h // 2, :],
                    ps_rep[s0 : s0 + 64, h * 64 : (h + 1) * 64],
                )
            st[b]["at"] = at
            y_sb = yp.tile([128, 4, N], BF16, tag="ysb", name=f"y_sb{b}")
            st[b]["y"] = y_sb

        def s_outT_pair(b, hp):
            """attn@v for head pair (2*hp, 2*hp+1), computed directly in the
            scrambled y layout: y[h*64+a, r*64+d] = sum_e v_h[e, a*16+r] *
            attn_h[d, e]. For each r (= n%16), lhsT is the column-strided v
            slice [e, a] and rhs is attn^T [e, d]; the matmul output block
            [64(a), 64(d)] IS a y tile. Even head runs in PE quadrant (0,0),
            odd head in (64,64), so the pair executes concurrently."""
            at, v_sb = st[b]["at"], st[b]["v"]
            y_sb = st[b]["y"]
            po_y = [
                ps_big.tile([128, 512], F32, tag="big", name=f"po{b}_{hp}_{sh}")
                for sh in range(2)
            ]
            # x columns are host-permuted to (r a) order, so v_sb's 64-column
            # block r*64:(r+1)*64 is exactly v_h[e, a*16+r] for a=0..63 --
            # contiguous, hence FWL-eligible weight loads.
            for r in range(16):
                sh, rr = r // 8, r % 8
                for hh in range(2):
                    s0 = hh * 64
                    nc.tensor.matmul(
                        po_y[sh][s0 : s0 + 64, rr * 64 : (rr + 1) * 64],
                        v_sb[s0 : s0 + 64, hp, r * 64 : (r + 1) * 64],
                        at[s0 : s0 + 64, hp, :],
                        start=True, stop=True,
                        tile_position=(s0, s0),
                    )
            nc.vector.tensor_copy(y_sb[:, hp, 0:512], po_y[0][:, :])
            nc.scalar.copy(y_sb[:, hp, 512:1024], po_y[1][:, :])

        def s_final_oc(b, oc, fine=False):
            y_sb = st[b]["y"]
            og = ogp.tile([128, N], F32, tag="og", name=f"og{b}_{oc}")
            osl = slice(oc * 128, (oc + 1) * 128)
            # both n-halves share the stationary wo chunk per cp; the two
            # PSUM groups close together so the copies drain in parallel
            pf = [
                ps_big.tile([128, 512], F32, tag="big", name=f"pf{b}_{oc}_{sh}")
                for sh in range(2)
            ]
            for cp in range(4):
                wc = w_sb["wo"][:, cp, oc * 128 : (oc + 1) * 128]
                for sh in range(2):
                    nc.tensor.matmul(
                        pf[sh][:, :], wc,
                        y_sb[:, cp, sh * 512 : (sh + 1) * 512],
                        start=(cp == 0), stop=(cp == 3),
                    )
            for sh in range(2):
                if not fine:
                    sl = slice(sh * 512, (sh + 1) * 512)
                    if sh == 0:
                        nc.vector.tensor_copy(og[:, sl], pf[sh][:, :])
                        nc.sync.dma_start(out_d[b, osl, sl], og[:, sl])
                    else:
                        nc.scalar.copy(og[:, sl], pf[sh][:, :])
                        nc.scalar.dma_start(out_d[b, osl, sl], og[:, sl])
                else:
                    # drain the kernel's last output with both copy engines
                    # and both DMA queues working in parallel: vector+sync
                    # take sh0, scalar+scalar-queue take sh1 (one trigger
                    # each, issued immediately after its copy)
                    sl = slice(sh * 512, (sh + 1) * 512)
                    if sh == 0:
                        nc.vector.tensor_copy(og[:, sl], pf[sh][:, :])
                        nc.sync.dma_start(out_d[b, osl, sl], og[:, sl])
                    else:
                        nc.scalar.copy(og[:, sl], pf[sh][:, :])
                        nc.scalar.dma_start(out_d[b, osl, sl], og[:, sl])

        # input loads, ordered by first use; every item split across both
        # HWDGE queues so the first q matmul starts ~8us and the projection
        # stream never starves.
        _load_w("wq", wq_d, nc.sync, nc.scalar)
        s_load_xa(0, split=True)
        _load_w("wk", wk_d, nc.sync, nc.scalar)
        s_load_xb(0)
        _load_w("wv", wv_d, nc.sync, nc.scalar)
        s_load_xa(1, split=False)
        s_load_xb(1)
        _load_w("wo", wo_d, nc.sync, nc.scalar)

        # two-batch software pipeline. attn@v phases (64-free matmuls, low
        # PE-array occupancy) are interleaved with 512-free projection work
        # so the HAM clock-gate never sees an idle window.
        s_proj_qk(0)
        s_scores(0)
        s_proj_v(0)
        s_attn_norm(0)
        s_proj_qk_alloc(1)
        for hp in range(4):
            s_outT_pair(0, hp)
            s_proj_qk_chunk(1, 2 * hp, "qk")
            s_proj_qk_chunk(1, 2 * hp + 1, "qk")
        s_scores(1)
        s_proj_v(1)
        s_attn_norm(1)
        for hp in range(4):
            s_outT_pair(1, hp)
            s_final_oc(0, hp)
        for oc in range(4):
            s_final_oc(1, oc, fine=(oc == 3))

    nc.compile()
    _split_excess_dma_waits(nc)
    return nc


_PROGRAM = None


def _get_program():
    global _PROGRAM
    if _PROGRAM is None:
        _PROGRAM = build_program()
    return _PROGRAM


def make_in_maps(x, Wq, Wk, Wv, Wo):
    bf = ml_dtypes.bfloat16
    x, Wq, Wk, Wv, Wo = (np.asarray(a, dtype=np.float32) for a in (x, Wq, Wk, Wv, Wo))
    # Permute spatial columns n = a*16+r -> position r*64+a. The q/k/v
    # projections and the q.k^T contraction are invariant to spatial order;
    # the permutation makes v_sb r-major so each attn@v stationary operand
    # is a contiguous 64-column block.
    x = x.reshape(16, C, 64, 16).transpose(0, 1, 3, 2).reshape(16, C, N)
    x = x.astype(bf)
    # xa: n-half-major layout [b, half, p, (q2 cc n')] -- per-partition
    # contiguous 4KB rows for the q/k projection loads.
    xa = np.ascontiguousarray(
        x.reshape(16, 4, 128, 2, 2, 256).transpose(0, 3, 2, 4, 1, 5).reshape(16, 2, 128, 2048)
    )
    x = np.ascontiguousarray(x)
    wqt = np.ascontiguousarray(Wq.T.astype(bf))
    wkt = np.ascontiguousarray(Wk.T.astype(bf))
    wvt = np.ascontiguousarray(Wv.T.astype(bf))
    wot = np.ascontiguousarray(Wo.T.astype(bf))
    in_maps = []
    for c in range(N_CORES):
        bs = slice(c * B_PER_CORE, (c + 1) * B_PER_CORE)
        in_maps.append(
            {
                "x": np.ascontiguousarray(x[bs]),
                "xa": np.ascontiguousarray(xa[bs]),
                "wqt": wqt,
                "wkt": wkt,
                "wvt": wvt,
                "wot": wot,
            }
        )
    return in_maps


def kernel(x, Wq, bq, Wk, bk, Wv, bv, Wo, bo, _trace=False):
    # biases are zeros by construction in this problem (spec fill="zeros");
    # they are not applied on-device.
    nc = _get_program()
    in_maps = make_in_maps(x, Wq, Wk, Wv, Wo)
    res = run_bass_kernel_spmd(nc, in_maps, list(range(N_CORES)), trace=_trace)
    outs = [np.asarray(res.results[c]["out"]) for c in range(N_CORES)]
    full = np.concatenate(outs, axis=0).reshape(16, C, 32, 32)
    if _trace:
        return full, res
    return full
